# revision 64
# baseline (speedup 1.0000x reference)
"""Bass/Trainium2 SPMD kernel for nn_Attention_18459769438488.

Sharding: 8 cores = 4 batches x 2 halves. Uniform graph; all per-core
variation is in host-prepared data (weight slices / permutations / zeroed
G matrices). Host sums the two per-batch partial projections at the end.

v2: exp-bound pipeline. Scores as 16-tile PE bursts straight from the
natural [4x32ch, N] layout (no q/k replication), attention*V col-tiled
(2 heads per psum bank), batched softmax normalization, and all
non-attention work (res/conv/lowpass/pool) paced as filler inside the
ACT-bound attention loop.
"""

import sys

for p in ("/opt/trn_rl_repo",):
    if p not in sys.path:
        sys.path.insert(0, p)

import numpy as np
import ml_dtypes

import concourse.bass as bass
import concourse.bacc as bacc
import concourse.mybir as mybir
import concourse.tile as tile

BF16 = mybir.dt.bfloat16
F32 = mybir.dt.float32
F8 = mybir.dt.float8e4
NPBF = ml_dtypes.bfloat16
NPF8 = ml_dtypes.float8_e4m3
WS = 32.0  # fp8 weight scale (keeps w*WS out of the e4m3 subnormal range)

B, DIM, HEADS, HH, WW = 4, 256, 8, 48, 48
N = HH * WW           # 2304
CH = DIM // HEADS     # 32
SCALE = CH ** -0.5
HP = HH + 6           # 54 padded
PADN = HP * HP + 8    # padded image cols (+ slack for shifted replicas)
MT = N // 128         # 18 key tiles
CHUNKS = [(0, 512), (512, 512), (1024, 512), (1536, 512), (2048, 256)]
NT = N // 128         # 18 n-tiles for proj
CONV_KS = (1, 3, 5, 7)


def _quads_for_k(k):
    """List of (dr, dc_base) quads covering all taps of a k x k kernel,
    4 consecutive dc per quad (zero-padded slots allowed)."""
    p = k // 2
    quads = []
    for dr in range(-p, p + 1):
        dc = -p
        while dc <= p:
            quads.append((dr, dc))
            dc += 4
    return quads


QUADS = {k: _quads_for_k(k) for k in CONV_KS}
NQUADS = sum(len(QUADS[k]) for k in CONV_KS)  # 1 + 3 + 10 + 14 = 28

# packed const-buffer column layout (all bf16, 128 rows; >128-row tensors are
# stored as consecutive 128-row column blocks). Critical weights first (they
# get their own early DMA), bulk after.
_SIZES = [("wqT", 2 * 256), ("wresT", 2 * 256), ("wresconvT", 2 * 128),
          ("wkTown", 2 * 128), ("wqTown", 2 * 128), ("wvT", 2 * 128),
          ("convW", 28 * 64), ("G", 2 * N), ("wpT", 5 * 256),
          ("selP", 128)]
C_OFF = {}
_o = 0
for _n, _w in _SIZES:
    C_OFF[_n] = _o
    _o += _w
CBUF_COLS = _o
CSPLIT = C_OFF["convW"]  # critical | bulk split


class _MT2:
    """Row-blocked (128) multi-tile wrapper with 2D slicing within one block."""

    def __init__(self, pool, rows, cols, dtype, name):
        self.rows, self.cols = rows, cols
        self.tiles = []
        for i in range((rows + 127) // 128):
            rw = min(128, rows - 128 * i)
            self.tiles.append(pool.tile([rw, cols], dtype, name=f"{name}_{i}"))

    def __getitem__(self, idx):
        rs, cs = idx
        r0 = rs.start or 0
        r1 = self.rows if rs.stop is None else rs.stop
        ti = r0 // 128
        assert r1 <= 128 * (ti + 1), (r0, r1)
        return self.tiles[ti][r0 - 128 * ti:r1 - 128 * ti, cs]


def build_nc():
    nc = bacc.Bacc("TRN2", target_bir_lowering=False, debug=False)

    cbuf_d = nc.declare_dram_parameter("cbuf", [128, CBUF_COLS], BF16, isOutput=False)
    idbuf_d = nc.declare_dram_parameter("idbuf", [128, 128], F32, isOutput=False)
    xT_d = nc.declare_dram_parameter("xT", [256, N], BF16, isOutput=False)
    out_d = nc.declare_dram_parameter("out", [N, 256], F32, isOutput=True)

    with tile.TileContext(nc) as tc:
        import contextlib

        ctx = contextlib.ExitStack()
        with ctx:
            consts = ctx.enter_context(tc.tile_pool(name="consts", bufs=1))
            big = ctx.enter_context(tc.tile_pool(name="big", bufs=1))

            # ---- constants: critical weights DMA first, bulk after ----
            cbuf = consts.tile([128, CBUF_COLS], BF16)
            identf = consts.tile([128, 128], F32)
            nc.sync.dma_start(out=cbuf[:, 0:CSPLIT], in_=cbuf_d[:, 0:CSPLIT])

            class _CV:
                def __init__(self, off, w):
                    self.off, self.w = off, w

                def __getitem__(self, idx):
                    rs, cs = idx
                    r0 = rs.start or 0
                    r1 = 128 * (r0 // 128 + 1) if rs.stop is None else rs.stop
                    ti = r0 // 128
                    c0 = cs.start or 0
                    c1 = self.w if cs.stop is None else cs.stop
                    return cbuf[r0 - 128 * ti:r1 - 128 * ti,
                                self.off + self.w * ti + c0:self.off + self.w * ti + c1]

            wqT = _CV(C_OFF["wqT"], 256)
            wresT = _CV(C_OFF["wresT"], 256)
            wresconvT = _CV(C_OFF["wresconvT"], 128)
            wkTown = _CV(C_OFF["wkTown"], 128)
            wqTown = _CV(C_OFF["wqTown"], 128)
            wvT = _CV(C_OFF["wvT"], 128)
            convW = _CV(C_OFF["convW"], NQUADS * 64)
            G = _CV(C_OFF["G"], 2 * N)
            wpT = _CV(C_OFF["wpT"], 256)
            selP = _CV(C_OFF["selP"], 128)

            # ---- big SBUF tensors ----
            qT = _MT2(big, 256, N, BF16, "qT")
            kTo = _MT2(big, 128, N, BF16, "kTo")
            qTo = _MT2(big, 128, N, BF16, "qTo")
            resT = _MT2(big, 256, N, BF16, "resT")
            res_conv = big.tile([128, N], BF16)  # conv input channels (4 x 32)
            vn = big.tile([128, MT * 132], BF16)  # [m-tile][head][33]
            attn_outT = big.tile([128, N], BF16)
            crpeT = _MT2(big, 256, N, BF16, "crpeT")
            lpT = _MT2(big, 256, N, BF16, "lpT")
            vpad4 = big.tile([128, 4 * PADN], BF16)  # [replica 32][group g]
            pvn = big.tile([128, 10 * 512], BF16)   # unnormalized pv, slot 2*ci+pair
            rbn = big.tile([128, 10 * 512], F32)    # 1/den in pair layout (rows 0-32/64-96)
            pooled6T = consts.tile([100, 256], BF16)
            dummy = consts.tile([1, 8], F32)

            # vn ones: only the stride-33 "ones" columns (v overwrites the rest)
            nc.vector.memset(
                vn[:, :].rearrange("p (m h c) -> p m h c", h=4, c=33)[:, :, :, 32:33],
                1.0)
            # warm the ACT exp table immediately (no DMA dependency)
            dummy_in = consts.tile([1, 8], F32)
            nc.vector.memset(dummy_in[:, :], 0.5)
            nc.scalar.activation(dummy[:, :], dummy_in[:, :],
                                 mybir.ActivationFunctionType.Exp)

            # xT DMA split by chunk so the first projections start early
            xT = []
            for t in range(2):
                xt = big.tile([128, N], BF16, name=f"xT{t}")
                xT.append(xt)
            for n0, W in CHUNKS:
                for t in range(2):
                    nc.sync.dma_start(out=xT[t][:, n0:n0 + W],
                                      in_=xT_d[128 * t:128 * (t + 1), n0:n0 + W])
            # non-critical constants after the attention-critical transfers
            nc.sync.dma_start(out=identf[:, :], in_=idbuf_d[:, :])
            nc.sync.dma_start(out=cbuf[:, CSPLIT:CBUF_COLS],
                              in_=cbuf_d[:, CSPLIT:CBUF_COLS])

            def proj_mm(ps_pool, dst, wt, wcols, evac, chunks=CHUNKS):
                for c0 in range(0, wcols, 128):
                    cw = min(128, wcols - c0)
                    for n0, W in chunks:
                        ps = ps_pool.tile([128, 512], F32, tag="ps")
                        for dt_ in range(2):
                            nc.tensor.matmul(
                                ps[0:cw, 0:W],
                                lhsT=wt[128 * dt_:128 * (dt_ + 1), c0:c0 + cw],
                                rhs=xT[dt_][:, n0:n0 + W],
                                start=(dt_ == 0), stop=(dt_ == 1))
                        evac(dst, c0, n0, W, cw, ps)

            def evac_copy(dst, c0, n0, W, cw, ps):
                nc.vector.tensor_copy(dst[c0:c0 + cw, n0:n0 + W], ps[0:cw, 0:W])

            def evac_relu(dst, c0, n0, W, cw, ps):
                nc.vector.tensor_relu(dst[c0:c0 + cw, n0:n0 + W], ps[0:cw, 0:W])

            with tc.tile_pool(name="ph1", bufs=2, space="PSUM") as ph1:
                # only the attention-critical q/k projections run upfront;
                # everything else is paced as filler inside the attention loop
                # per-chunk interleave so the first score burst can start as
                # soon as the first xT slice lands
                for n0, W in CHUNKS:
                    proj_mm(ph1, kTo, wkTown, 128, evac_copy, chunks=[(n0, W)])
                    proj_mm(ph1, qTo, wqTown, 128, evac_copy, chunks=[(n0, W)])

            # ================= FILLER UNITS =================
            # Emitted paced inside the attention loop; all use the shared
            # 2-buf psum pool `fps`.
            fillers = []

            def make_proj_unit(fps, dst, wt, c0, cw, n0, W, evac):
                def unit():
                    ps = fps.tile([128, 512], F32, tag="fps")
                    for dt_ in range(2):
                        nc.tensor.matmul(
                            ps[0:cw, 0:W],
                            lhsT=wt[128 * dt_:128 * (dt_ + 1), c0:c0 + cw],
                            rhs=xT[dt_][:, n0:n0 + W],
                            start=(dt_ == 0), stop=(dt_ == 1))
                    evac(dst, c0, n0, W, cw, ps)
                return unit

            def make_vn_unit(fps, mi):
                # v natural: per m-tile (128 tokens, 128 ch) -> vn strided [head][33]
                def unit():
                    ps = fps.tile([128, 512], F32, tag="fps")
                    for dt_ in range(2):
                        nc.tensor.matmul(
                            ps[:, 0:128],
                            lhsT=xT[dt_][:, 128 * mi:128 * (mi + 1)],
                            rhs=wvT[128 * dt_:128 * (dt_ + 1), :],
                            start=(dt_ == 0), stop=(dt_ == 1))
                    src = ps[:, 0:128].rearrange("p (h c) -> p h c", h=4)
                    dst = vn[:, 132 * mi:132 * (mi + 1)].rearrange(
                        "p (h c) -> p h c", h=4)[:, :, 0:32]
                    nc.vector.tensor_copy(dst, src)
                return unit

            def make_vpad_unit(r):
                def unit():
                    for g in range(4):
                        c0 = g * PADN + 3 * HP + 3 - r
                        dst = vpad4[32 * r:32 * (r + 1), c0:c0 + HH * HP]
                        dst = dst.rearrange("p (row col) -> p row col",
                                            row=HH, col=HP)[:, :, 0:HH]
                        src = res_conv[32 * g:32 * (g + 1), :].rearrange(
                            "p (row col) -> p row col", row=HH)
                        nc.vector.tensor_copy(dst, src)
                return unit

            def make_memset_unit(ap, val):
                def unit():
                    nc.vector.memset(ap, val)
                return unit

            def make_pool_unit(fps, t):
                def unit():
                    pooled6 = consts.tile([128, 36], F32, name=f"pooled6_{t}")
                    src = resT[128 * t:128 * (t + 1), :].rearrange(
                        "p (br i bc j) -> p br bc i j", br=6, i=8, bc=6, j=8)
                    nc.vector.tensor_reduce(
                        pooled6[:, :], src, axis=mybir.AxisListType.XY,
                        op=mybir.AluOpType.add)
                    tp = fps.tile([36, 128], F32, tag="fps")
                    nc.tensor.transpose(tp[:, :], pooled6[:, :], identf[:, :])
                    nc.vector.tensor_copy(pooled6T[0:36, 128 * t:128 * (t + 1)], tp[:, :])
                    nc.vector.tensor_copy(pooled6T[64:100, 128 * t:128 * (t + 1)], tp[:, :])
                return unit

            def make_conv_unit(fps, nr0, gi):
                # one conv group chain; consecutive units land on alternating
                # col positions + psum slots so the PE overlaps them
                def unit():
                    s = gi % 2
                    cps = fps.tile([128, 384], F32, tag="fps")
                    k = CONV_KS[gi]
                    quads = QUADS[k]
                    qoff = sum(len(QUADS[CONV_KS[g]]) for g in range(gi))
                    for t, (dr, dc_base) in enumerate(quads):
                        o = gi * PADN + (3 + nr0 + dr) * HP + 3 + dc_base
                        rhs = vpad4[:, o:o + 8 * HP]
                        rhs = rhs.rearrange("p (row rest) -> p row rest", row=8)[:, :, 0:HH]
                        nc.tensor.matmul(
                            cps[64 * s:64 * (s + 1), :],
                            lhsT=convW[:, 64 * (qoff + t):64 * (qoff + t + 1)],
                            rhs=rhs,
                            start=(t == 0), stop=(t == len(quads) - 1),
                            tile_position=(0, 64 * s),
                            skip_group_check=True)
                    tt = gi // 2
                    r0 = 64 * (gi % 2)
                    nc.vector.tensor_mul(
                        crpeT[128 * tt + r0:128 * tt + r0 + 64, 48 * nr0:48 * (nr0 + 8)],
                        qT[64 * gi:64 * (gi + 1), 48 * nr0:48 * (nr0 + 8)],
                        cps[64 * s:64 * (s + 1), :])
                return unit

            def make_lp_unit(fps, n0, W, pr):
                # lowpass groups (2*pr, 2*pr+1): row-group tiled (pooled6T rows
                # 0:36 / 64:100), written to disjoint psum partition halves
                def unit():
                    lps = fps.tile([128, 512], F32, tag="fps")
                    for s, gi in enumerate((2 * pr, 2 * pr + 1)):
                        rg = 64 * (gi % 2)
                        nc.tensor.matmul(
                            lps[64 * s:64 * s + 64, 0:W],
                            lhsT=pooled6T[rg:rg + 36, 64 * gi:64 * (gi + 1)],
                            rhs=G[rg:rg + 36, N * (gi // 2) + n0:N * (gi // 2) + n0 + W],
                            start=True, stop=True,
                            tile_position=(rg, 64 * s),
                            skip_group_check=True)
                    for s, gi in enumerate((2 * pr, 2 * pr + 1)):
                        tt = gi // 2
                        r0 = 64 * (gi % 2)
                        nc.vector.tensor_relu(
                            lpT[128 * tt + r0:128 * tt + r0 + 64, n0:n0 + W],
                            lps[64 * s:64 * s + 64, 0:W])
                return unit

            # ================= ATTENTION (exp-bound pipeline) =================
            with (
                tc.tile_pool(name="sps", bufs=2, space="PSUM") as sps_pool,
                tc.tile_pool(name="pvp", bufs=1, space="PSUM") as pv_pool,
                tc.tile_pool(name="fps", bufs=2, space="PSUM") as fps,
                tc.tile_pool(name="expP", bufs=4) as exp_pool,
                tc.tile_pool(name="yout", bufs=3) as yout_pool,
            ):
                def make_projout_unit(nt):
                    def unit():
                        yps = fps.tile([128, 512], F32, tag="fps")
                        srcs = [
                            attn_outT[:, 128 * nt:128 * (nt + 1)],
                            crpeT[0:128, 128 * nt:128 * (nt + 1)],
                            crpeT[128:256, 128 * nt:128 * (nt + 1)],
                            lpT[0:128, 128 * nt:128 * (nt + 1)],
                            lpT[128:256, 128 * nt:128 * (nt + 1)],
                        ]
                        for bi, src in enumerate(srcs):
                            nc.tensor.matmul(
                                yps[:, 0:256], lhsT=src,
                                rhs=wpT[128 * bi:128 * (bi + 1), :],
                                start=(bi == 0), stop=(bi == 4))
                        ybuf = yout_pool.tile([128, 256], F32, tag="ybuf")
                        nc.vector.tensor_copy(ybuf[:, :], yps[:, 0:256])
                        nc.sync.dma_start(out=out_d[128 * nt:128 * (nt + 1), :],
                                          in_=ybuf[:, :])
                    return unit

                # build filler queue (order respects data deps)
                for mi in range(8):
                    fillers.append(make_vn_unit(fps, mi))
                fillers.append(make_memset_unit(pvn[:, :], 0.0))
                for mi in range(8, MT):
                    fillers.append(make_vn_unit(fps, mi))
                for g in range(4):
                    fillers.append(make_memset_unit(
                        vpad4[:, g * PADN:(g + 1) * PADN], 0.0))
                for t in range(2):
                    for n0, W in CHUNKS:
                        fillers.append(make_proj_unit(
                            fps, resT, wresT, 128 * t, 128, n0, W, evac_relu))
                for n0, W in CHUNKS:
                    fillers.append(make_proj_unit(
                        fps, res_conv, wresconvT, 0, 128, n0, W, evac_relu))
                for r in range(4):
                    fillers.append(make_vpad_unit(r))
                for t in range(2):
                    fillers.append(make_pool_unit(fps, t))
                for t in range(2):
                    for n0, W in CHUNKS:
                        fillers.append(make_proj_unit(
                            fps, qT, wqT, 128 * t, 128, n0, W, evac_copy))
                for nr0 in range(0, HH, 8):
                    for gi in range(4):
                        fillers.append(make_conv_unit(fps, nr0, gi))
                for n0, W in CHUNKS:
                    for pr in range(2):
                        fillers.append(make_lp_unit(fps, n0, W, pr))
                for nt in range(NT - 2):  # last 2 tiles need chunk-4 norm: tail
                    fillers.append(make_projout_unit(nt))
                fillers = list(fillers)
                fidx = 0
                n_units = len(fillers)
                n_iters = len(CHUNKS) * MT

                def emit_filler(it):
                    nonlocal fidx
                    # pace the queue uniformly, with a small head start so the
                    # vn units stay ahead of the pv consumers
                    want = min(n_units, 2 + (it + 1) * n_units // n_iters)
                    while fidx < want:
                        fillers[fidx]()
                        fidx += 1

                it = 0
                prev = None  # (tiles, ci, W, pva, pvb, mi)
                pending_norm = None

                def emit_pv(prev):
                    tiles, ci, W, pva, pvb, mi = prev
                    for h in range(4):
                        pair, s = h // 2, h % 2
                        expP = tiles[pair]
                        rhs = expP[:, 512 * s:512 * s + W]
                        pv = pva if pair == 0 else pvb
                        nc.tensor.matmul(
                            pv[64 * s:64 * s + 33, 0:W],
                            lhsT=vn[:, 132 * mi + 33 * h:132 * mi + 33 * (h + 1)],
                            rhs=rhs,
                            start=(mi == 0), stop=(mi == MT - 1),
                            tile_position=(0, 64 * s),
                            skip_group_check=True)

                for ci, (n0, W) in enumerate(CHUNKS):
                    pva = pv_pool.tile([128, 512], F32, tag="pva")
                    pvb = pv_pool.tile([128, 512], F32, tag="pvb")
                    # exp span: heads sit at 512-col offsets (own psum bank per
                    # row tile); for the 256-wide tail one call covers
                    # [0:768] incl. 256 junk cols (never read downstream).
                    espan = 1024 if W == 512 else 512 + W
                    for mi in range(MT):
                        exp_tiles = []
                        sps_list = []
                        for pair in (0, 1):
                            sps = sps_pool.tile([128, 1024], F32, tag="sps")
                            for s in (0, 1):
                                h = 2 * pair + s
                                nc.tensor.matmul(
                                    sps[0:128, 512 * s:512 * s + W],
                                    lhsT=kTo[32 * h:32 * (h + 1),
                                             128 * mi:128 * (mi + 1)],
                                    rhs=qTo[32 * h:32 * (h + 1), n0:n0 + W],
                                    start=True, stop=True,
                                    tile_position=(32 * h, 0),
                                    skip_group_check=True)
                            sps_list.append(sps)
                        # pv of the previous iteration (keeps PE fed while ACT runs)
                        if prev is not None:
                            emit_pv(prev)
                        if pending_norm is not None and mi == 1:
                            pending_norm()
                            pending_norm = None
                        # exp evacuations
                        for sps in sps_list:
                            expP = exp_pool.tile([128, 1024], BF16, tag="expP")
                            nc.scalar.activation(
                                expP[:, 0:espan], sps[:, 0:espan],
                                mybir.ActivationFunctionType.Exp, scale=SCALE)
                            exp_tiles.append(expP)
                        emit_filler(it)
                        it += 1
                        prev = (exp_tiles, ci, W, pva, pvb, mi)
                    # flush last pv of the chunk + evacuate unnormalized pv;
                    # the normalization itself (selector den broadcast + fast
                    # reciprocal + muls) is deferred into the next chunk so it
                    # doesn't block the next chunk's bursts in the PE queue
                    emit_pv(prev)
                    prev = None
                    for pair, pv in ((0, pva), (1, pvb)):
                        nc.vector.tensor_copy(
                            pvn[:, (2 * ci + pair) * 512:(2 * ci + pair) * 512 + W],
                            pv[:, 0:W])

                    def make_norm(ci=ci, n0=n0, W=W):
                        def norm():
                            for pair in (0, 1):
                                slot = 2 * ci + pair
                                rbden = fps.tile([128, 512], F32, tag="fps")
                                nc.tensor.matmul(
                                    rbden[:, 0:W],
                                    lhsT=selP[0:128, 0:128],
                                    rhs=pvn[:, slot * 512:slot * 512 + W],
                                    start=True, stop=True,
                                    skip_group_check=True)
                                nc.vector.reciprocal_approx_fast(
                                    out=rbn[:, slot * 512:slot * 512 + W],
                                    in_=rbden[:, 0:W])
                            for h in range(4):
                                pair, s = h // 2, h % 2
                                slot = 2 * ci + pair
                                nc.vector.tensor_mul(
                                    attn_outT[32 * h:32 * (h + 1), n0:n0 + W],
                                    pvn[64 * s:64 * s + 32, slot * 512:slot * 512 + W],
                                    rbn[64 * s:64 * s + 32, slot * 512:slot * 512 + W])
                        return norm
                    pending_norm = make_norm()

                pending_norm()
                pending_norm = None
                # flush remaining fillers, then the last two output tiles
                while fidx < n_units:
                    fillers[fidx]()
                    fidx += 1
                for nt in range(NT - 2, NT):
                    make_projout_unit(nt)()
    nc.finalize()
    return nc


# =====================  HOST SIDE  =====================

def _upsample_matrix(s, H=48):
    U = np.zeros((s * s, H * H), np.float64)
    if s == 1:
        U[0, :] = 1.0
        return U
    c = np.arange(H) * (s - 1) / (H - 1)
    c0 = np.floor(c).astype(int)
    c1 = np.minimum(c0 + 1, s - 1)
    w = c - c0
    for r in range(H):
        for x in range(H):
            n = r * H + x
            U[c0[r] * s + c0[x], n] += (1 - w[r]) * (1 - w[x])
            U[c0[r] * s + c1[x], n] += (1 - w[r]) * w[x]
            U[c1[r] * s + c0[x], n] += w[r] * (1 - w[x])
            U[c1[r] * s + c1[x], n] += w[r] * w[x]
    return U


def _reduce_matrix(s):
    # R (s^2, 36): pool6 grid (6x6) -> pool_s grid, uniform means
    R = np.zeros((s * s, 36), np.float64)
    b = 6 // s
    for a in range(s):
        for c_ in range(s):
            for i in range(b):
                for j in range(b):
                    R[a * s + c_, (a * b + i) * 6 + (c_ * b + j)] = 1.0 / (b * b)
    return R


def _g_matrices():
    gs = []
    for s in (1, 2, 3, 6):
        R = _reduce_matrix(s)          # (s^2, 36)
        U = _upsample_matrix(s)        # (s^2, 2304)
        gs.append((R.T @ U) / 64.0)    # (36, 2304); /64 folds the 8x8 block sum
    return gs  # list of (36, 2304)


def _constrain_np(kern, k):
    C = kern.shape[0]
    ctr = (k * k) // 2
    full = np.concatenate(
        [kern[:, :, :ctr], -np.ones((C, C, 1), kern.dtype), kern[:, :, ctr:]],
        axis=2)
    return full.reshape(C, C, k, k)


def _host_prep(x, wq, wkv, wres, wproj, kern3, kern5, kern7):
    """Returns list of 8 in_maps."""
    wk = wkv[0:256]
    wv = wkv[256:512]
    kfull = {
        1: np.eye(64, dtype=np.float32).reshape(64, 64, 1, 1),
        3: _constrain_np(np.asarray(kern3, np.float32), 3),
        5: _constrain_np(np.asarray(kern5, np.float32), 5),
        7: _constrain_np(np.asarray(kern7, np.float32), 7),
    }
    gs = _g_matrices()

    # den broadcast selector (pair layout): out[0:32] = pvn[32], out[64:96] = pvn[96]
    selP = np.zeros((128, 128), np.float32)
    selP[32, 0:32] = 1.0
    selP[96, 64:96] = 1.0

    in_maps = []
    for b in range(B):
        for j in range(2):
            own = slice(128 * j, 128 * (j + 1))
            # conv weight quads
            convW = np.zeros((128, NQUADS * 64), np.float32)
            qoff = 0
            for gi, k in enumerate(CONV_KS):
                kf = kfull[k][:, 32 * j:32 * (j + 1)]  # (64out, 32in, k, k)
                p = k // 2
                for (dr, dc_base) in QUADS[k]:
                    for slot in range(4):
                        dc = dc_base + slot
                        if dc <= p:
                            convW[32 * slot:32 * (slot + 1), 64 * qoff:64 * (qoff + 1)] = \
                                kf[:, :, dr + p, dc + p].T
                    qoff += 1
            # G with non-owned groups zeroed; packed (100, 2N):
            # group g at rows 64*(g%2).. cols N*(g//2)..
            G = np.zeros((100, 2 * N), np.float32)
            for gi in range(4):
                if gi // 2 == j:
                    G[64 * (gi % 2):64 * (gi % 2) + 36, N * (gi // 2):N * (gi // 2) + N] = gs[gi]
            # conv input channels: group gi uses res channels 64gi+32j .. +32
            rows = np.concatenate(
                [np.arange(64 * gi + 32 * j, 64 * gi + 32 * j + 32) for gi in range(4)])
            wpT = np.concatenate([
                wproj[:, own].T,
                wproj[:, 0:128].T, wproj[:, 128:256].T,
                wproj[:, 0:128].T, wproj[:, 128:256].T], axis=0)  # (640, 256)

            def _pack(M):
                M = np.asarray(M, np.float32)
                R = M.shape[0]
                blocks = []
                for r0 in range(0, R, 128):
                    blk = M[r0:r0 + 128]
                    if blk.shape[0] < 128:
                        blk = np.concatenate(
                            [blk, np.zeros((128 - blk.shape[0], M.shape[1]), np.float32)])
                    blocks.append(blk)
                return np.concatenate(blocks, axis=1)

            parts = {
                "wqT": _pack(wq.T), "wresT": _pack(wres.T),
                "wresconvT": _pack(wres[rows].T),
                "wkTown": _pack(wk[own].T),
                "wqTown": _pack(wq[own].T),
                "wvT": _pack(wv[own].T), "convW": _pack(convW),
                "G": _pack(G), "wpT": _pack(wpT), "selP": _pack(selP),
            }
            cb = np.zeros((128, CBUF_COLS), np.float32)
            for nmk, _w in _SIZES:
                p = parts[nmk]
                assert p.shape == (128, _w), (nmk, p.shape, _w)
                cb[:, C_OFF[nmk]:C_OFF[nmk] + _w] = p
            xT = np.ascontiguousarray(np.asarray(x[b], np.float32).T).astype(NPBF)
            in_maps.append({
                "cbuf": cb.astype(NPBF),
                "idbuf": np.eye(128, dtype=np.float32),
                "xT": xT,
            })
    return in_maps


_CACHED_NC = None


def kernel(x, H, W, wq, wkv, wres, wproj, bproj, kern3, kern5, kern7, **kw):
    global _CACHED_NC
    from concourse.bass_utils import run_bass_kernel_spmd

    x = np.asarray(x, np.float32)
    wq = np.asarray(wq, np.float32)
    wkv = np.asarray(wkv, np.float32)
    wres = np.asarray(wres, np.float32)
    wproj = np.asarray(wproj, np.float32)
    bproj = np.asarray(bproj, np.float32)

    in_maps = _host_prep(x, wq, wkv, wres, wproj, kern3, kern5, kern7)
    if _CACHED_NC is None:
        _CACHED_NC = build_nc()
    res = run_bass_kernel_spmd(_CACHED_NC, in_maps, core_ids=list(range(8)))
    outs = [np.asarray(res.results[i]["out"], np.float32) for i in range(8)]
    y = np.stack([outs[2 * b] + outs[2 * b + 1] for b in range(B)]) + bproj
    return y.astype(np.float32)
